# revision 82
# baseline (speedup 1.0000x reference)
"""TRN2 Bass kernel for nn_Attention_90460601189287.

Causal multi-head attention (B=2, N=2048, D=1024, H=16) with spectral-norm
(power-iteration) scaled qkv/proj dense layers, on 8 NeuronCores.

Sharding: tensor-parallel over heads. Core c owns heads {2c, 2c+1}: it gets
the matching 128 columns of each of W_qkv's q/k/v blocks and the matching
128 rows of W_proj, computes attention for its heads over the full batch,
and produces a partial y = x_att @ W_proj_rows (fp16). The host sums the 8
partials in fp32 (the gather step for row-sharded matmul).

The tiny spectral-norm power-iteration scales (identical math to the
reference: sigma = ||W^T normalize(W u)||) are computed on host in fp32 and
folded into the weights / a single per-side scale gamma.

Device program (SPMD; per-core weight slices), per 512-token window w:
  A: qkv^T = W^T x^T, with x^T provided by the host (fp8 for q/k, fp16 for
     v) so no on-device transposes of x are needed. q,k run as fp8
     DoubleRow matmuls (2 d-chunks per pass, 0.5 cyc/row); v in fp16.
     q^T/k^T are stored as fp8 with a zero second slot so the attention
     S matmul can also use DoubleRow; v^T is PE-transposed into V-natural
     tiles whose cols 64:127 are ones (the PV matmul then also emits the
     softmax denominator for free).
  B: per (head, 2-k-block group): S^T = K Q^T via one fp8 DoubleRow matmul
     per k-block into a 2-bank PSUM group; one Exp activation per group
     (no max pass / shift: scores are O(1) so exp() is in fp16 range);
     causal triangle mask multiply on diagonal blocks; O^T accumulated in
     PSUM with the denominator on partitions 64:127; normalize via
     reciprocal+mult.
  C: y_partial = x_att^T-blocks @ W_proj, staged via Pool copies, fp16 DMA.

Engines are in-order, so stage A(w+1) and C(w-1) ops are interleaved into
stage B(w)'s exp-latency bubbles explicitly (ACT is the critical engine:
it runs only the 80 grouped exps).
"""
from contextlib import ExitStack

import numpy as np

import concourse.bass as bass
import concourse.mybir as mybir
from concourse.bass_utils import run_bass_kernel_spmd
from concourse.tile import TileContext

F32 = mybir.dt.float32
F16 = mybir.dt.float16
F8 = mybir.dt.float8e4

N_CORES = 8
BATCH = 2
NTOK = 4096      # flattened b*n
D = 1024
NH = 2           # heads per core
HD = 64
B = 2
NSEQ = 2048
WQ = 512         # token window
NW = NTOK // WQ
NWB = NSEQ // WQ
KB = 128
BETA = 8.0       # host-side fp8 weight pre-scale (keeps W in fp8e4m3 range)

DR = mybir.MatmulPerfMode.DoubleRow
EXP = mybir.ActivationFunctionType.Exp
MUL = mybir.AluOpType.mult


# ---------------------------------------------------------------------------
# Workaround: this walrus build accepts at most ONE sync wait per
# instruction. Hoist extra waits onto single-wait NOPs inserted before.
# ---------------------------------------------------------------------------
def _split_sync_waits(nc, max_waits=1):
    for f in nc.m.functions:
        for blk in f.blocks:
            insts = blk.instructions
            out = []
            changed = False
            for inst in insts:
                si = inst.sync_info
                waits = list(si.on_wait) if si is not None else []
                if len(waits) > max_waits:
                    extra = waits[:-max_waits]
                    for i in range(0, len(extra), max_waits):
                        nop = mybir.InstNoOp(name=f"I-{nc.next_id()}", ins=[],
                                             outs=[], engine=inst.engine)
                        nop.sync_info = mybir.SyncInfo(
                            on_wait=extra[i:i + max_waits], on_update=[])
                        nc.register_instruction(nop, overwrite=True)
                        out.append(nop)
                    si.on_wait = waits[-max_waits:]
                    inst.sync_info = si
                    changed = True
                out.append(inst)
            if changed:
                blk.instructions = out


class _TileContextSplit(TileContext):
    def __exit__(self, exc_type, exc_value, traceback):
        ret = super().__exit__(exc_type, exc_value, traceback)
        if exc_type is None:
            _split_sync_waits(self.nc)
        return ret


def declare_params(nc):
    xb = nc.declare_dram_parameter("xb", [D, NTOK], F16, isOutput=False)
    x8 = nc.declare_dram_parameter("x8", [D, NTOK], F8, isOutput=False)
    w8qk = nc.declare_dram_parameter("w8qk", [D, 2 * NH * HD], F8,
                                     isOutput=False)
    wvh = nc.declare_dram_parameter("wvh", [D, NH * HD], F16, isOutput=False)
    wph = nc.declare_dram_parameter("wph", [NH * HD, D], F16, isOutput=False)
    # col 0: gamma; cols 1..128: -50*strict-upper-triangle (transposed mask
    # addend for the diagonal S blocks); cols 129..256: identity
    cst = nc.declare_dram_parameter("cst", [128, 1 + 2 * KB], F16,
                                    isOutput=False)
    y = nc.declare_dram_parameter("y", [NTOK, D], F16, isOutput=True)
    return xb, x8, w8qk, wvh, wph, cst, y


def _build_body(nc, tc):
    xb, x8, w8qk, wvh, wph, cst, y = declare_params(nc)

    ctx = ExitStack()
    with ctx:
        singles = ctx.enter_context(tc.tile_pool(name="singles", bufs=1))

        # --- constants / weights to SBUF (all on the SP/HWDGE queue; the
        # Pool DGE path costs ~1us of descriptor generation per transfer) ---
        cst_sb = singles.tile([128, 1 + 2 * KB], F16)
        triM_sb = cst_sb[:, 1:1 + KB]
        ident_sb = cst_sb[:, 1 + KB:1 + 2 * KB]
        gam_sb = singles.tile([128, 1], F32)

        w8qk_sb = singles.tile([128, 8, 2 * NH * HD], F8)
        nc.sync.dma_start(out=w8qk_sb[:],
                          in_=w8qk.rearrange("(c p) m -> p c m", p=128))
        w8q_sb = w8qk_sb[:, :, 0:NH * HD]
        w8k_sb = w8qk_sb[:, :, NH * HD:2 * NH * HD]
        wvh_sb = singles.tile([128, 8, NH * HD], F16)
        wph_sb = singles.tile([128, D], F16)

        # --- persistent per-window tiles ---
        # q^T/k^T as fp8 with a zero slot (dim1) so the S matmul can run in
        # DoubleRow mode (2 contraction tiles per pass; slot 1 contributes 0)
        qT8 = [singles.tile([128, 2, WQ], F8, name=f"qT8_{w}")
               for w in range(NW)]
        kT8 = [singles.tile([128, 2, WQ], F8, name=f"kT8_{w}")
               for w in range(NW)]
        xaw = [singles.tile([128, WQ], F16, name=f"xa_{w}") for w in range(NW)]
        # V natural layout per (head, batch, k-window): [128 k, 4 kb, v|ones]
        vnat = [[[singles.tile([128, 4, 2 * HD], F16, name=f"vn_{h}_{b}_{g}")
                  for g in range(NWB)] for b in range(B)] for h in range(NH)]

        warm = singles.tile([128, KB], F16)

        # zero the fp8 DoubleRow padding slots and the all-ones denominator
        # columns once, on Pool (window-major so window 0 unblocks first)
        nc.gpsimd.memset(warm[:], 0.0)
        for w in range(NW):
            b, g = divmod(w, NWB)
            nc.gpsimd.memset(qT8[w][:, 1, :], 0.0)
            nc.gpsimd.memset(kT8[w][:, 1, :], 0.0)
            for h in range(NH):
                nc.gpsimd.memset(vnat[h][b][g][:, :, HD:2 * HD], 1.0)

        # --- pools ---
        # PSUM budget (8 banks): s 2x2 + o 2 + aux 2 = 8. The "aux" ring
        # carries all short-lived stage A/C accumulators (q, k, v-natural,
        # proj partials) so they never contend with the S-group ring.
        ps = ctx.enter_context(tc.tile_pool(name="ps", bufs=1, space="PSUM"))
        x8_pool = ctx.enter_context(tc.tile_pool(name="x8p", bufs=3))
        xb_pool = ctx.enter_context(tc.tile_pool(name="xbp", bufs=3))
        a_pool = ctx.enter_context(tc.tile_pool(name="apool", bufs=6))
        den_pool = ctx.enter_context(tc.tile_pool(name="denpool", bufs=3))
        y_pool = ctx.enter_context(tc.tile_pool(name="ypool", bufs=5))

        x8_pre = {}
        xb_pre = {}

        def load_x8(w):
            ws = slice(w * WQ, (w + 1) * WQ)
            x8_t = x8_pool.tile([128, 8, WQ], F8, tag="x8", name="x8_t")
            nc.sync.dma_start(
                out=x8_t[:],
                in_=x8[:, ws].rearrange("(c p) n -> p c n", p=128))
            x8_pre[w] = x8_t

        def load_xb(w):
            ws = slice(w * WQ, (w + 1) * WQ)
            xb_t = xb_pool.tile([128, 8, WQ], F16, tag="xb", name="xb_t")
            nc.sync.dma_start(
                out=xb_t[:],
                in_=xb[:, ws].rearrange("(c p) n -> p c n", p=128))
            xb_pre[w] = xb_t

        def load_x(w):
            if w in x8_pre:
                return
            load_x8(w)
            load_xb(w)

        # PE p-state warmup: the array only reaches full clock after ~3us of
        # continuous execution, so burn dummy matmuls on a zeroed tile while
        # the first x8 window is still in flight
        for _ in range(26):
            wp_ps = ps.tile([128, 2, WQ], F32, tag="s", name="warm_ps",
                            bufs=2)
            nc.tensor.matmul(wp_ps[:, 0, 0:KB], warm[:], warm[:],
                             start=True, stop=True)

        load_x8(0)
        nc.sync.dma_start(out=cst_sb[:], in_=cst[:])
        nc.vector.tensor_copy(gam_sb[:], cst_sb[:, 0:1])
        nc.sync.dma_start(out=wvh_sb[:],
                          in_=wvh.rearrange("(c p) m -> p c m", p=128))
        load_xb(0)
        load_x8(1)
        nc.sync.dma_start(out=wph_sb[:], in_=wph[:])
        load_xb(1)

        # ---- Stage A, as lists of (pe_ns_estimate, closure) so the emitter
        # can pace them into stage B's exp-latency bubbles. q/k (fp8
        # DoubleRow) only need x8; v (natural layout, f16) only needs xb. ----
        def ops_A_qk(w):
            st = {}
            ops = []

            def op_load():
                if w not in x8_pre:
                    load_x(w)
            ops.append((0, op_load))

            def mk_qk(w8_sb, dst, on_act=False):
                def op_mm():
                    p = ps.tile([128, WQ], F32, tag="aux", name="qkp",
                                bufs=2)
                    for cp in range(4):
                        nc.tensor.matmul(
                            p[:], w8_sb[:, 2 * cp:2 * cp + 2, :],
                            x8_pre[w][:, 2 * cp:2 * cp + 2, :],
                            start=(cp == 0), stop=(cp == 3), perf_mode=DR)
                    st['p'] = p

                def op_cp():
                    if on_act:
                        nc.scalar.mul(dst[:, 0, :], st['p'][:], gam_sb)
                    else:
                        nc.vector.tensor_scalar_mul(dst[:, 0, :], st['p'][:],
                                                    gam_sb)
                return [(430, op_mm), (60, op_cp)]

            ops += mk_qk(w8q_sb, qT8[w])
            # window 0's k copy runs on the (then idle) ACT engine so the
            # first S matmul isn't gated on two serial DVE copies
            ops += mk_qk(w8k_sb, kT8[w], on_act=(w == 0))
            return ops

        def ops_A_v(w):
            # v in natural [token, head-col] layout via xb as the stationary
            # operand: no transposes needed for the PV lhsT
            b, g = divmod(w, NWB)
            ops = []

            def mk_v(half):
                def op():
                    vn_ps = ps.tile([128, 2, NH * HD], F32, tag="aux",
                                    name="vn_ps", bufs=2)
                    for tb in range(2 * half, 2 * half + 2):
                        for c in range(8):
                            nc.tensor.matmul(
                                vn_ps[:, tb - 2 * half, :],
                                xb_pre[w][:, c, tb * KB:(tb + 1) * KB],
                                wvh_sb[:, c, :],
                                start=(c == 0), stop=(c == 7))
                    for h in range(NH):
                        nc.vector.tensor_copy(
                            vnat[h][b][g][:, 2 * half:2 * half + 2, 0:HD],
                            vn_ps[:, :, h * HD:(h + 1) * HD])
                    if half == 1:
                        x8_pre.pop(w)
                        xb_pre.pop(w)
                return op
            ops.append((900, mk_v(0)))
            ops.append((900, mk_v(1)))
            return ops

        # ---- Stage C for one token window: proj partials, 4 n-blocks.
        # The last window's copies alternate DVE/ACT and DMA per half so the
        # post-last-exp tail drains two engines wide. ----
        # Tail variant of stage C: the final window's staging drains after
        # the last exp with every engine idle, so batch each n-block through
        # a free 2-bank "s" tile and alternate the big copies DVE/ACT.
        def ops_C_tail(w):
            ops = []
            for nb_loc in range(4):
                def op_mm(nb_loc=nb_loc):
                    yp = ps.tile([128, 2, WQ], F32, tag="s", name="ypt",
                                 bufs=2)
                    for cc in range(2):
                        nc.tensor.matmul(
                            yp[:, cc, :],
                            xaw[w][:, nb_loc * 128:(nb_loc + 1) * 128],
                            wph_sb[:, cc * WQ:(cc + 1) * WQ],
                            start=True, stop=True)
                    y_sb = y_pool.tile([128, D], F16, tag="y", name="y_sbt")
                    nc.vector.tensor_copy(y_sb[:, 0:WQ], yp[:, 0, :])
                    nc.scalar.copy(y_sb[:, WQ:D], yp[:, 1, :])
                    nb = 4 * w + nb_loc
                    nc.sync.dma_start(out=y[nb * 128:(nb + 1) * 128, :],
                                      in_=y_sb[:])
                ops.append((450, op_mm))
            return ops

        def ops_C(w):
            st = {}
            ops = []
            for nb_loc in range(4):
                for cc in range(2):
                    def op_mm(nb_loc=nb_loc, cc=cc):
                        yp = ps.tile([128, WQ], F32, tag="aux", name="yp",
                                     bufs=2)
                        nc.tensor.matmul(
                            yp[:],
                            xaw[w][:, nb_loc * 128:(nb_loc + 1) * 128],
                            wph_sb[:, cc * WQ:(cc + 1) * WQ],
                            start=True, stop=True)
                        nb = 4 * w + nb_loc
                        if cc == 0:
                            st['y'] = y_pool.tile([128, D], F16, tag="y",
                                                  name="y_sb")
                        y_sb = st['y']
                        nc.vector.tensor_copy(
                            y_sb[:, cc * WQ:(cc + 1) * WQ], yp[:])
                        if cc == 1:
                            nc.sync.dma_start(
                                out=y[nb * 128:(nb + 1) * 128, :],
                                in_=y_sb[:])
                    ops.append((250, op_mm))
            return ops

        # ---- Stage B: globally software-pipelined over unit pairs. Each
        # pair's PVs (and, for the last pair of a (b,g), the reciprocal
        # normalization) are deferred into the NEXT pair's emission so the
        # in-order PE queue never parks on a just-issued exp — including
        # across (b,g) boundaries. ----
        pend_pv = [None]

        # Global pair-stream filler schedule: pair p (0..39) across all
        # (b,g) gets sched[p] closures; stage-C closures drain from a global
        # queue at a rate that spreads them over the remaining pairs.
        n_pairs_w = [2 * (divmod(w, NWB)[1] + 1) for w in range(NW)]
        pair_start = [sum(n_pairs_w[:w]) for w in range(NW + 1)]
        total_pairs = pair_start[NW]
        sched = [[] for _ in range(total_pairs)]
        carry_c = []
        pair_idx = [0]

        def place(items, lo, hi):
            # spread items evenly over pairs [lo, hi), preserving order
            n = len(items)
            span = max(1, hi - lo)
            for i, it in enumerate(items):
                p = min(hi - 1, lo + i * span // n)
                sched[max(0, p)].append(it)

        def stage_B(b, g):
            w = b * NWB + g
            n_pairs = 2 * (g + 1)
            st = {}

            def emit_pv(u, a_ts):
                kw, j = divmod(u, 2)
                diag = (kw == g)
                if u == 0:
                    st['o'] = [ps.tile([128, WQ], F32, tag="o",
                                       name=f"o_ps{h}", bufs=2)
                               for h in range(NH)]
                o_ps = st['o']
                for h in range(NH):
                    a_t = a_ts[h]
                    for t in range(2):
                        kloc = 2 * j + t
                        sq = kloc * KB if diag else 0
                        nc.tensor.matmul(
                            o_ps[h][:, sq:WQ] if sq else o_ps[h][:],
                            vnat[h][b][kw][:, kloc, :],
                            a_t[:, t, sq:WQ] if sq else a_t[:, t, :],
                            start=(u == 0 and t == 0),
                            stop=(u == n_pairs - 1 and t == 1))
                if u == n_pairs - 1:
                    if w == NW - 1:
                        # final window: normalize per token-half so the tail
                        # proj can start before the second half is done
                        for tk in range(2):
                            ts = slice(tk * 256, (tk + 1) * 256)
                            for h in range(NH):
                                hs = slice(h * HD, (h + 1) * HD)
                                den = den_pool.tile([HD, 256], F32,
                                                    tag="denh", name="denh")
                                nc.vector.reciprocal(
                                    den[:], o_ps[h][HD:2 * HD, ts])
                                nc.vector.tensor_tensor(
                                    out=xaw[w][hs, ts],
                                    in0=o_ps[h][0:HD, ts],
                                    in1=den[:], op=MUL)
                    else:
                        for h in range(NH):
                            hs = slice(h * HD, (h + 1) * HD)
                            den = den_pool.tile([HD, WQ], F32, tag="den",
                                                name="den")
                            nc.vector.reciprocal(den[:],
                                                 o_ps[h][HD:2 * HD, :])
                            nc.vector.tensor_tensor(out=xaw[w][hs, :],
                                                    in0=o_ps[h][0:HD, :],
                                                    in1=den[:], op=MUL)

            for u in range(n_pairs):
                kw, j = divmod(u, 2)
                kwin = b * NWB + kw
                # columns below 2j*KB of a diagonal group are causally dead:
                # skip them in both the S matmuls and the exp
                c0 = 2 * j * KB if kw == g else 0
                a_ts = [None, None]
                for h in range(NH):
                    hs = slice(h * HD, (h + 1) * HD)
                    s_grp = ps.tile([128, 2, WQ], F32, tag="s", name="s_grp",
                                    bufs=2)
                    for t in range(2):
                        kloc = 2 * j + t
                        diag_t = (kw == g)
                        nc.tensor.matmul(
                            s_grp[:, t, c0:WQ],
                            kT8[kwin][hs, :, kloc * KB:(kloc + 1) * KB],
                            qT8[w][hs, :, c0:WQ],
                            start=True, stop=not diag_t, perf_mode=DR)
                        if diag_t:
                            # causal mask: add -50 to the strictly-masked
                            # band; exp underflows those to exactly 0 in f16
                            bq = kloc * KB
                            nc.tensor.matmul(
                                s_grp[:, t, bq:bq + KB],
                                triM_sb, ident_sb,
                                start=False, stop=True)
                    a_t = a_pool.tile([128, 2, WQ], F16, tag="a", name="a_t")
                    nc.scalar.activation(a_t[:, :, c0:WQ],
                                         s_grp[:, :, c0:WQ], EXP)
                    a_ts[h] = a_t
                # previous pair's PVs (and possibly the previous window's
                # finale) FIRST: stage-C closures behind it read xaw, and the
                # tile dep tracker only orders reads after already-emitted
                # writes
                if pend_pv[0] is not None:
                    pend_pv[0]()
                pend_pv[0] = (lambda u=u, a_ts=a_ts: emit_pv(u, a_ts))
                # filler into the exp latency bubble: this pair's scheduled
                # stage-A closures, then stage-C at the global drain rate
                p = pair_idx[0]
                for _, fn in sched[p]:
                    fn()
                want_c = -(-len(carry_c) // max(1, total_pairs - p))
                for _ in range(min(want_c, 2, len(carry_c))):
                    carry_c.pop(0)[1]()
                pair_idx[0] = p + 1

        def riffle(a, c):
            out = []
            n = max(len(a), len(c))
            for i in range(n):
                if i < len(a):
                    out.append(a[i])
                if i < len(c):
                    out.append(c[i])
            return out

        # ---- software-pipelined emission ----
        # A_qk(0) up front (x8-only, lands early); v(0) in the first pairs
        # so the PE queue never parks on the larger xb DMA. A(w) is placed
        # in the ~7 pairs preceding B(w)'s start (crossing (b,g) boundaries
        # so short windows don't choke on filler).
        for _, op in ops_A_qk(0):
            op()
        place(ops_A_v(0), 0, 2)
        for w in range(1, NW):
            a = ops_A_qk(w) + ops_A_v(w)
            # the x load (first op, no PE work) goes early; the matmuls no
            # earlier than the previous window's start so a pending x DMA
            # never head-of-line-blocks the in-order PE queue
            pl = max(0, pair_start[w] - 11)
            place(a[:1], pl, pl + 1)
            lo = max(pair_start[w] - 8, pair_start[w - 1] + 1)
            place(a[1:], lo, max(pair_start[w] - 2, lo + 1))
        for w in range(NW):
            b, g = divmod(w, NWB)
            if w >= 1:
                carry_c.extend(ops_C(w - 1))
            stage_B(b, g)
        pend_pv[0]()
        for _, op in carry_c:
            op()
        for _, op in ops_C_tail(NW - 1):
            op()


def _host_scales(W_qkv, u_qkv, sigma_qkv, W_proj, u_proj, sigma_proj):
    """Power-iteration spectral norm in fp32, exactly as the reference:
    v = normalize(W u); sigma = ||W^T v||."""
    def sig(W, u):
        v = (W @ u).astype(np.float32)
        v = v / np.float32(np.linalg.norm(v))
        u2 = (W.T @ v).astype(np.float32)
        return np.float32(np.linalg.norm(u2))
    c_qkv = np.float32(sigma_qkv[0]) / sig(W_qkv, u_qkv)
    c_proj = np.float32(sigma_proj[0]) / sig(W_proj, u_proj)
    return np.float32(c_qkv), np.float32(c_proj)


def _make_tri():
    p = np.arange(128)[:, None]
    q = np.arange(KB)[None, :]
    return (q >= p).astype(np.float32)


def make_in_maps(batch, W_qkv, u_qkv, sigma_qkv, W_proj, u_proj, sigma_proj):
    import ml_dtypes
    f16 = np.float16
    f8 = ml_dtypes.float8_e4m3
    batch = np.asarray(batch, np.float32)
    W_qkv = np.asarray(W_qkv, np.float32)
    u_qkv = np.asarray(u_qkv, np.float32)
    sigma_qkv = np.asarray(sigma_qkv, np.float32)
    W_proj = np.asarray(W_proj, np.float32)
    u_proj = np.asarray(u_proj, np.float32)
    sigma_proj = np.asarray(sigma_proj, np.float32)
    c_qkv, c_proj = _host_scales(W_qkv, u_qkv, sigma_qkv,
                                 W_proj, u_proj, sigma_proj)
    xT = np.ascontiguousarray(batch.reshape(NTOK, D).T)
    xb_h = xT.astype(f16)
    x8_h = xT.astype(f8)
    # S needs scale c^2/sqrt(hd); gamma is the per-side share applied at the
    # q^T/k^T PSUM->SBUF copies (fp8 weights carry a BETA pre-scale to stay
    # in e4m3's normal range)
    # (gamma*BETA)^2 == c^2 / sqrt(hd)  =>  gamma = c / (BETA * hd^(1/4))
    gamma = np.float32(c_qkv / (BETA * HD ** 0.25))
    a = np.arange(128)
    triM = np.where(a[:, None] < a[None, :], -50.0, 0.0).astype(np.float32)
    ident = np.eye(128, dtype=np.float32)
    cst = np.concatenate(
        [np.full((128, 1), gamma, np.float32), triM, ident],
        axis=1).astype(f16)
    cst = np.ascontiguousarray(cst)
    in_maps = []
    for c in range(N_CORES):
        cs = slice(128 * c, 128 * (c + 1))
        w8q_h = np.ascontiguousarray((W_qkv[:, cs] * BETA).astype(f8))
        w8k_h = np.ascontiguousarray(
            (W_qkv[:, 1024 + 128 * c:1024 + 128 * (c + 1)] * BETA).astype(f8))
        wvh_h = np.ascontiguousarray(
            (W_qkv[:, 2048 + 128 * c:2048 + 128 * (c + 1)] * c_qkv)
            .astype(f16))
        wph_h = np.ascontiguousarray((W_proj[cs, :] * c_proj).astype(f16))
        in_maps.append({
            "xb": xb_h,
            "x8": x8_h,
            "w8qk": np.ascontiguousarray(
                np.concatenate([w8q_h, w8k_h], axis=1)),
            "wvh": wvh_h,
            "wph": wph_h,
            "cst": cst,
        })
    return in_maps


_NC_CACHE = None


def build_nc():
    global _NC_CACHE
    if _NC_CACHE is None:
        nc = bass.Bass("TRN2", target_bir_lowering=False, debug=False,
                       num_devices=N_CORES)
        with _TileContextSplit(nc) as tc:
            _build_body(nc, tc)
        _NC_CACHE = nc
    return _NC_CACHE


def kernel(batch, W_qkv, u_qkv, sigma_qkv, W_proj, u_proj, sigma_proj):
    in_maps = make_in_maps(batch, W_qkv, u_qkv, sigma_qkv,
                           W_proj, u_proj, sigma_proj)
    nc = build_nc()
    res = run_bass_kernel_spmd(nc, in_maps, list(range(N_CORES)))
    y = np.zeros((NTOK, D), np.float32)
    for c in range(N_CORES):
        y += res.results[c]["y"].astype(np.float32)
    return y.reshape(BATCH, NSEQ, D)


# revision 100
# speedup vs baseline: 3.7398x; 3.7398x over previous
"""TRN2 Bass kernel for nn_Attention_90460601189287.

Causal multi-head attention (B=2, N=2048, D=1024, H=16) with spectral-norm
(power-iteration) scaled qkv/proj dense layers, on 8 NeuronCores.

Sharding: tensor-parallel over heads. Core c owns heads {2c, 2c+1}: it gets
the matching 128 columns of each of W_qkv's q/k/v blocks and the matching
128 rows of W_proj, computes attention for its heads over the full batch,
and produces a partial y = x_att @ W_proj_rows (fp16). The host sums the 8
partials in fp32 (the gather step for row-sharded matmul).

The tiny spectral-norm power-iteration scales (identical math to the
reference: sigma = ||W^T normalize(W u)||) are computed on host in fp32 and
folded into the weights / a single per-side scale gamma.

Device program (SPMD; per-core weight slices), per 512-token window w:
  A: qkv^T = W^T x^T, with x^T provided by the host (fp8 for q/k, fp16 for
     v) so no on-device transposes of x are needed. q,k run as fp8
     DoubleRow matmuls (2 d-chunks per pass, 0.5 cyc/row); v in fp16.
     q^T/k^T are stored as fp8 with a zero second slot so the attention
     S matmul can also use DoubleRow; v^T is PE-transposed into V-natural
     tiles whose cols 64:127 are ones (the PV matmul then also emits the
     softmax denominator for free).
  B: per (head, 2-k-block group): S^T = K Q^T via one fp8 DoubleRow matmul
     per k-block into a 2-bank PSUM group; one Exp activation per group
     (no max pass / shift: scores are O(1) so exp() is in fp16 range);
     causal triangle mask multiply on diagonal blocks; O^T accumulated in
     PSUM with the denominator on partitions 64:127; normalize via
     reciprocal+mult.
  C: y_partial = x_att^T-blocks @ W_proj, staged via Pool copies, fp16 DMA.

Engines are in-order, so stage A(w+1) and C(w-1) ops are interleaved into
stage B(w)'s exp-latency bubbles explicitly (ACT is the critical engine:
it runs only the 80 grouped exps).
"""
from contextlib import ExitStack

import numpy as np

import concourse.bass as bass
import concourse.mybir as mybir
from concourse.bass_utils import run_bass_kernel_spmd
from concourse.tile import TileContext

F32 = mybir.dt.float32
F16 = mybir.dt.float16
F8 = mybir.dt.float8e4

N_CORES = 8
BATCH = 2
NTOK = 4096      # flattened b*n
D = 1024
NH = 2           # heads per core
HD = 64
B = 2
NSEQ = 2048
WQ = 512         # token window
NW = NTOK // WQ
NWB = NSEQ // WQ
KB = 128
BETA = 8.0       # host-side fp8 weight pre-scale (keeps W in fp8e4m3 range)

DR = mybir.MatmulPerfMode.DoubleRow
EXP = mybir.ActivationFunctionType.Exp
MUL = mybir.AluOpType.mult
DIV = mybir.AluOpType.divide


# ---------------------------------------------------------------------------
# Workaround: this walrus build accepts at most ONE sync wait per
# instruction. Hoist extra waits onto single-wait NOPs inserted before.
# ---------------------------------------------------------------------------
def _split_sync_waits(nc, max_waits=1):
    for f in nc.m.functions:
        for blk in f.blocks:
            insts = blk.instructions
            out = []
            changed = False
            for inst in insts:
                si = inst.sync_info
                waits = list(si.on_wait) if si is not None else []
                if len(waits) > max_waits:
                    extra = waits[:-max_waits]
                    for i in range(0, len(extra), max_waits):
                        nop = mybir.InstNoOp(name=f"I-{nc.next_id()}", ins=[],
                                             outs=[], engine=inst.engine)
                        nop.sync_info = mybir.SyncInfo(
                            on_wait=extra[i:i + max_waits], on_update=[])
                        nc.register_instruction(nop, overwrite=True)
                        out.append(nop)
                    si.on_wait = waits[-max_waits:]
                    inst.sync_info = si
                    changed = True
                out.append(inst)
            if changed:
                blk.instructions = out


class _TileContextSplit(TileContext):
    def __exit__(self, exc_type, exc_value, traceback):
        ret = super().__exit__(exc_type, exc_value, traceback)
        if exc_type is None:
            _split_sync_waits(self.nc)
        return ret


def declare_params(nc):
    xb = nc.declare_dram_parameter("xb", [D, NTOK], F16, isOutput=False)
    x8 = nc.declare_dram_parameter("x8", [D, NTOK], F8, isOutput=False)
    w8qk = nc.declare_dram_parameter("w8qk", [D, 2 * NH * HD], F8,
                                     isOutput=False)
    wvh = nc.declare_dram_parameter("wvh", [D, NH * HD], F16, isOutput=False)
    wph = nc.declare_dram_parameter("wph", [NH * HD, D], F16, isOutput=False)
    # col 0: gamma; cols 1..128: -50*strict-upper-triangle (transposed mask
    # addend for the diagonal S blocks); cols 129..256: identity
    cst = nc.declare_dram_parameter("cst", [128, 1 + 2 * KB], F16,
                                    isOutput=False)
    y = nc.declare_dram_parameter("y", [NTOK, D], F16, isOutput=True)
    return xb, x8, w8qk, wvh, wph, cst, y


def _build_body(nc, tc):
    xb, x8, w8qk, wvh, wph, cst, y = declare_params(nc)

    ctx = ExitStack()
    with ctx:
        singles = ctx.enter_context(tc.tile_pool(name="singles", bufs=1))

        # --- constants / weights to SBUF (all on the SP/HWDGE queue; the
        # Pool DGE path costs ~1us of descriptor generation per transfer) ---
        cst_sb = singles.tile([128, 1 + 2 * KB], F16)
        triM_sb = cst_sb[:, 1:1 + KB]
        ident_sb = cst_sb[:, 1 + KB:1 + 2 * KB]
        gam_sb = singles.tile([128, 1], F32)

        w8qk_sb = singles.tile([128, 8, 2 * NH * HD], F8)
        nc.sync.dma_start(out=w8qk_sb[:],
                          in_=w8qk.rearrange("(c p) m -> p c m", p=128))
        w8q_sb = w8qk_sb[:, :, 0:NH * HD]
        w8k_sb = w8qk_sb[:, :, NH * HD:2 * NH * HD]
        wvh_sb = singles.tile([128, 8, NH * HD], F16)
        wph_sb = singles.tile([128, D], F16)

        # --- persistent per-window tiles ---
        # q^T/k^T as fp8 with a zero slot (dim1) so the S matmul can run in
        # DoubleRow mode (2 contraction tiles per pass; slot 1 contributes 0)
        qT8 = [singles.tile([128, 2, WQ], F8, name=f"qT8_{w}")
               for w in range(NW)]
        kT8 = [singles.tile([128, 2, WQ], F8, name=f"kT8_{w}")
               for w in range(NW)]
        xaw = [singles.tile([128, WQ], F16, name=f"xa_{w}") for w in range(NW)]
        # V natural layout per (head, batch, k-window): [128 k, 4 kb, v|ones]
        vnat = [[[singles.tile([128, 4, 2 * HD], F16, name=f"vn_{h}_{b}_{g}")
                  for g in range(NWB)] for b in range(B)] for h in range(NH)]

        warm = singles.tile([128, KB], F16)

        # zero the fp8 DoubleRow padding slots and the all-ones denominator
        # columns once, on Pool (window-major so window 0 unblocks first)
        nc.gpsimd.memset(warm[:], 0.0)
        for w in range(NW):
            b, g = divmod(w, NWB)
            nc.gpsimd.memset(qT8[w][:, 1, :], 0.0)
            nc.gpsimd.memset(kT8[w][:, 1, :], 0.0)
            for h in range(NH):
                nc.gpsimd.memset(vnat[h][b][g][:, :, HD:2 * HD], 1.0)

        # --- pools ---
        # PSUM budget (8 banks): s 2x2 + o 2 + aux 2 = 8. The "aux" ring
        # carries all short-lived stage A/C accumulators (q, k, v-natural,
        # proj partials) so they never contend with the S-group ring.
        ps = ctx.enter_context(tc.tile_pool(name="ps", bufs=1, space="PSUM"))
        x8_pool = ctx.enter_context(tc.tile_pool(name="x8p", bufs=3))
        xb_pool = ctx.enter_context(tc.tile_pool(name="xbp", bufs=3))
        a_pool = ctx.enter_context(tc.tile_pool(name="apool", bufs=6))
        den_pool = ctx.enter_context(tc.tile_pool(name="denpool", bufs=3))
        y_pool = ctx.enter_context(tc.tile_pool(name="ypool", bufs=5))

        x8_pre = {}
        xb_pre = {}

        def load_x8(w, split=False):
            ws = slice(w * WQ, (w + 1) * WQ)
            x8_t = x8_pool.tile([128, 8, WQ], F8, tag="x8", name="x8_t")
            src = x8[:, ws].rearrange("(c p) n -> p c n", p=128)
            if split:
                nc.sync.dma_start(out=x8_t[:, 0:4, :], in_=src[:, 0:4, :])
                nc.sync.dma_start(out=x8_t[:, 4:8, :], in_=src[:, 4:8, :])
            else:
                nc.sync.dma_start(out=x8_t[:], in_=src)
            x8_pre[w] = x8_t

        def load_xb(w):
            ws = slice(w * WQ, (w + 1) * WQ)
            xb_t = xb_pool.tile([128, 8, WQ], F16, tag="xb", name="xb_t")
            nc.sync.dma_start(
                out=xb_t[:],
                in_=xb[:, ws].rearrange("(c p) n -> p c n", p=128))
            xb_pre[w] = xb_t

        def load_x(w):
            if w in x8_pre:
                return
            load_x8(w)
            load_xb(w)

        # PE p-state warmup: the array only reaches full clock after ~3us of
        # continuous execution, so burn dummy matmuls on a zeroed tile while
        # the first x8 window is still in flight
        for _ in range(26):
            wp_ps = ps.tile([128, 2, WQ], F32, tag="s", name="warm_ps",
                            bufs=2)
            nc.tensor.matmul(wp_ps[:, 0, 0:KB], warm[:], warm[:],
                             start=True, stop=True)

        load_x8(0, split=True)
        nc.sync.dma_start(out=cst_sb[:], in_=cst[:])
        nc.vector.tensor_copy(gam_sb[:], cst_sb[:, 0:1])
        nc.sync.dma_start(out=wvh_sb[:],
                          in_=wvh.rearrange("(c p) m -> p c m", p=128))
        load_xb(0)
        load_x8(1)
        nc.sync.dma_start(out=wph_sb[:], in_=wph[:])
        load_xb(1)

        # ---- Stage A, as lists of (pe_ns_estimate, closure) so the emitter
        # can pace them into stage B's exp-latency bubbles. q/k (fp8
        # DoubleRow) only need x8; v (natural layout, f16) only needs xb. ----
        def ops_A_qk(w):
            st = {}
            ops = []

            def op_load():
                if w not in x8_pre:
                    load_x(w)
            ops.append((0, op_load))

            def op_qk_mm():
                pq = ps.tile([128, WQ], F32, tag="aux", name="qp", bufs=2)
                pk = ps.tile([128, WQ], F32, tag="aux", name="kp", bufs=2)
                st['pq'], st['pk'] = pq, pk
                for cp in range(4):
                    nc.tensor.matmul(
                        pq[:], w8q_sb[:, 2 * cp:2 * cp + 2, :],
                        x8_pre[w][:, 2 * cp:2 * cp + 2, :],
                        start=(cp == 0), stop=(cp == 3), perf_mode=DR)
                    nc.tensor.matmul(
                        pk[:], w8k_sb[:, 2 * cp:2 * cp + 2, :],
                        x8_pre[w][:, 2 * cp:2 * cp + 2, :],
                        start=(cp == 0), stop=(cp == 3), perf_mode=DR)

            def op_qc():
                if w == 0:
                    # window 0's q copy runs on the (then idle) ACT engine
                    # so the first S isn't gated on two serial DVE copies
                    nc.scalar.mul(qT8[w][:, 0, :], st['pq'][:], gam_sb)
                else:
                    nc.vector.tensor_scalar_mul(qT8[w][:, 0, :], st['pq'][:],
                                                gam_sb)

            def op_kc():
                # split so the first S pair (k columns 0:256) starts after
                # the first half lands
                nc.vector.tensor_scalar_mul(kT8[w][:, 0, 0:256],
                                            st['pk'][:, 0:256], gam_sb)
                nc.vector.tensor_scalar_mul(kT8[w][:, 0, 256:WQ],
                                            st['pk'][:, 256:WQ], gam_sb)
            ops += [(860, op_qk_mm), (60, op_qc), (60, op_kc)]
            return ops

        def ops_A_v(w):
            # v in natural [token, head-col] layout via xb as the stationary
            # operand: no transposes needed for the PV lhsT
            b, g = divmod(w, NWB)
            ops = []

            def mk_v(half):
                def op():
                    vn_ps = ps.tile([128, 2, NH * HD], F32, tag="aux",
                                    name="vn_ps", bufs=2)
                    for tb in range(2 * half, 2 * half + 2):
                        for c in range(8):
                            nc.tensor.matmul(
                                vn_ps[:, tb - 2 * half, :],
                                xb_pre[w][:, c, tb * KB:(tb + 1) * KB],
                                wvh_sb[:, c, :],
                                start=(c == 0), stop=(c == 7))
                    for h in range(NH):
                        nc.vector.tensor_copy(
                            vnat[h][b][g][:, 2 * half:2 * half + 2, 0:HD],
                            vn_ps[:, :, h * HD:(h + 1) * HD])
                    if half == 1:
                        x8_pre.pop(w)
                        xb_pre.pop(w)
                return op
            ops.append((900, mk_v(0)))
            ops.append((900, mk_v(1)))
            return ops

        # ---- Stage C for one token window: proj partials, 4 n-blocks.
        # The last window's copies alternate DVE/ACT and DMA per half so the
        # post-last-exp tail drains two engines wide. ----
        # Tail variant of stage C: the final window's staging drains after
        # the last exp with every engine idle, so batch each n-block through
        # a free 2-bank "s" tile and alternate the big copies DVE/ACT.
        def ops_C_tail(w):
            ops = []
            for nb_loc in range(4):
                def op_mm(nb_loc=nb_loc):
                    yp = ps.tile([128, 2, WQ], F32, tag="s", name="ypt",
                                 bufs=2)
                    for cc in range(2):
                        nc.tensor.matmul(
                            yp[:, cc, :],
                            xaw[w][:, nb_loc * 128:(nb_loc + 1) * 128],
                            wph_sb[:, cc * WQ:(cc + 1) * WQ],
                            start=True, stop=True)
                    y_sb = y_pool.tile([128, D], F16, tag="y", name="y_sbt")
                    nc.vector.tensor_copy(y_sb[:, 0:WQ], yp[:, 0, :])
                    nc.scalar.copy(y_sb[:, WQ:D], yp[:, 1, :])
                    nb = 4 * w + nb_loc
                    nc.sync.dma_start(out=y[nb * 128:(nb + 1) * 128, :],
                                      in_=y_sb[:])
                ops.append((450, op_mm))
            return ops

        def ops_C(w):
            st = {}
            ops = []
            for nb_loc in range(4):
                for cc in range(2):
                    def op_mm(nb_loc=nb_loc, cc=cc):
                        yp = ps.tile([128, WQ], F32, tag="aux", name="yp",
                                     bufs=2)
                        nc.tensor.matmul(
                            yp[:],
                            xaw[w][:, nb_loc * 128:(nb_loc + 1) * 128],
                            wph_sb[:, cc * WQ:(cc + 1) * WQ],
                            start=True, stop=True)
                        nb = 4 * w + nb_loc
                        if cc == 0:
                            st['y'] = y_pool.tile([128, D], F16, tag="y",
                                                  name="y_sb")
                        y_sb = st['y']
                        nc.vector.tensor_copy(
                            y_sb[:, cc * WQ:(cc + 1) * WQ], yp[:])
                        if cc == 1:
                            nc.sync.dma_start(
                                out=y[nb * 128:(nb + 1) * 128, :],
                                in_=y_sb[:])
                    ops.append((250, op_mm))
            return ops

        # ---- Stage B: globally software-pipelined over unit pairs. Each
        # pair's PVs (and, for the last pair of a (b,g), the reciprocal
        # normalization) are deferred into the NEXT pair's emission so the
        # in-order PE queue never parks on a just-issued exp — including
        # across (b,g) boundaries. ----
        pend_pv = [None]

        # Global pair-stream filler schedule: pair p (0..39) across all
        # (b,g) gets sched[p] closures; stage-C closures drain from a global
        # queue at a rate that spreads them over the remaining pairs.
        # processing order of the B windows: start with (0,1) before (0,0)
        # so the opening run of exps is long enough to hide stage-A chains
        order = [0, 1, 2, 3, 4, 5, 6, 7]
        pos_of = {w: i for i, w in enumerate(order)}
        n_pairs_w = [2 * (divmod(w, NWB)[1] + 1) for w in order]
        pair_start = [sum(n_pairs_w[:i]) for i in range(NW + 1)]
        total_pairs = pair_start[NW]
        sched = [[] for _ in range(total_pairs)]
        carry_c = []
        pair_idx = [0]

        def place(items, lo, hi):
            # spread items evenly over pairs [lo, hi), preserving order
            n = len(items)
            span = max(1, hi - lo)
            for i, it in enumerate(items):
                p = min(hi - 1, lo + i * span // n)
                sched[max(0, p)].append(it)

        def stage_B(b, g):
            w = b * NWB + g
            n_pairs = 2 * (g + 1)
            st = {}

            def emit_pv(u, a_ts):
                kw, j = divmod(u, 2)
                diag = (kw == g)
                if u == 0:
                    st['o'] = [ps.tile([128, WQ], F32, tag="o",
                                       name=f"o_ps{h}", bufs=2)
                               for h in range(NH)]
                o_ps = st['o']
                for h in range(NH):
                    a_t = a_ts[h]
                    for t in range(2):
                        kloc = 2 * j + t
                        sq = kloc * KB if diag else 0
                        nc.tensor.matmul(
                            o_ps[h][:, sq:WQ] if sq else o_ps[h][:],
                            vnat[h][b][kw][:, kloc, :],
                            a_t[:, t, sq:WQ] if sq else a_t[:, t, :],
                            start=(u == 0 and t == 0),
                            stop=(u == n_pairs - 1 and t == 1))
                if u == n_pairs - 1:
                    if w == NW - 1:
                        # final window: normalize per token-half so the tail
                        # proj can start before the second half is done
                        for tk in range(2):
                            ts = slice(tk * 256, (tk + 1) * 256)
                            for h in range(NH):
                                hs = slice(h * HD, (h + 1) * HD)
                                den = den_pool.tile([HD, 256], F32,
                                                    tag="denh", name="denh")
                                nc.vector.reciprocal(
                                    den[:], o_ps[h][HD:2 * HD, ts])
                                nc.vector.tensor_tensor(
                                    out=xaw[w][hs, ts],
                                    in0=o_ps[h][0:HD, ts],
                                    in1=den[:], op=MUL)
                    else:
                        for h in range(NH):
                            hs = slice(h * HD, (h + 1) * HD)
                            den = den_pool.tile([HD, WQ], F32, tag="den",
                                                name="den")
                            nc.vector.reciprocal(den[:],
                                                 o_ps[h][HD:2 * HD, :])
                            nc.vector.tensor_tensor(out=xaw[w][hs, :],
                                                    in0=o_ps[h][0:HD, :],
                                                    in1=den[:], op=MUL)

            for u in range(n_pairs):
                kw, j = divmod(u, 2)
                kwin = b * NWB + kw
                # columns below 2j*KB of a diagonal group are causally dead:
                # skip them in both the S matmuls and the exp
                c0 = 2 * j * KB if kw == g else 0
                a_ts = [None, None]
                for h in range(NH):
                    hs = slice(h * HD, (h + 1) * HD)
                    s_grp = ps.tile([128, 2, WQ], F32, tag="s", name="s_grp",
                                    bufs=2)
                    for t in range(2):
                        kloc = 2 * j + t
                        diag_t = (kw == g)
                        nc.tensor.matmul(
                            s_grp[:, t, c0:WQ],
                            kT8[kwin][hs, :, kloc * KB:(kloc + 1) * KB],
                            qT8[w][hs, :, c0:WQ],
                            start=True, stop=not diag_t, perf_mode=DR)
                        if diag_t:
                            # causal mask: add -50 to the strictly-masked
                            # band; exp underflows those to exactly 0 in f16
                            bq = kloc * KB
                            nc.tensor.matmul(
                                s_grp[:, t, bq:bq + KB],
                                triM_sb, ident_sb,
                                start=False, stop=True)
                    a_t = a_pool.tile([128, 2, WQ], F16, tag="a", name="a_t")
                    nc.scalar.activation(a_t[:, :, c0:WQ],
                                         s_grp[:, :, c0:WQ], EXP)
                    a_ts[h] = a_t
                # previous pair's PVs (and possibly the previous window's
                # finale) FIRST: stage-C closures behind it read xaw, and the
                # tile dep tracker only orders reads after already-emitted
                # writes
                if pend_pv[0] is not None:
                    pend_pv[0]()
                pend_pv[0] = (lambda u=u, a_ts=a_ts: emit_pv(u, a_ts))
                # filler into the exp latency bubble: this pair's scheduled
                # stage-A closures, then stage-C at the global drain rate
                p = pair_idx[0]
                for _, fn in sched[p]:
                    fn()
                want_c = -(-len(carry_c) // max(1, total_pairs - p))
                for _ in range(min(want_c, 3, len(carry_c))):
                    carry_c.pop(0)[1]()
                pair_idx[0] = p + 1

        def riffle(a, c):
            out = []
            n = max(len(a), len(c))
            for i in range(n):
                if i < len(a):
                    out.append(a[i])
                if i < len(c):
                    out.append(c[i])
            return out

        # ---- software-pipelined emission ----
        # A_qk(0) up front (x8-only, lands early); v(0) in the first pairs
        # so the PE queue never parks on the larger xb DMA. A(w) is placed
        # in the ~7 pairs preceding B(w)'s start (crossing (b,g) boundaries
        # so short windows don't choke on filler).
        for _, op in ops_A_qk(0):
            op()
        place(ops_A_v(0), 0, 2)
        for w in range(1, NW):
            a = ops_A_qk(w) + ops_A_v(w)
            sp = pair_start[pos_of[w]]
            prev_sp = pair_start[pos_of[w] - 1]
            # the x load (first op, no PE work) goes early; the matmuls no
            # earlier than the previous window's start so a pending x DMA
            # never head-of-line-blocks the in-order PE queue
            pl = max(0, sp - 11)
            place(a[:1], pl, pl + 1)
            lo = max(sp - 9, prev_sp + 1)
            place(a[1:], lo, max(sp - 4, lo + 1))
        done_b = []
        for w in order:
            b, g = divmod(w, NWB)
            if done_b:
                carry_c.extend(ops_C(done_b[-1]))
            stage_B(b, g)
            done_b.append(w)
        pend_pv[0]()
        for _, op in carry_c:
            op()
        for _, op in ops_C_tail(NW - 1):
            op()


def _host_scales(W_qkv, u_qkv, sigma_qkv, W_proj, u_proj, sigma_proj):
    """Power-iteration spectral norm in fp32, exactly as the reference:
    v = normalize(W u); sigma = ||W^T v||."""
    def sig(W, u):
        v = (W @ u).astype(np.float32)
        v = v / np.float32(np.linalg.norm(v))
        u2 = (W.T @ v).astype(np.float32)
        return np.float32(np.linalg.norm(u2))
    c_qkv = np.float32(sigma_qkv[0]) / sig(W_qkv, u_qkv)
    c_proj = np.float32(sigma_proj[0]) / sig(W_proj, u_proj)
    return np.float32(c_qkv), np.float32(c_proj)


def _make_tri():
    p = np.arange(128)[:, None]
    q = np.arange(KB)[None, :]
    return (q >= p).astype(np.float32)


def make_in_maps(batch, W_qkv, u_qkv, sigma_qkv, W_proj, u_proj, sigma_proj):
    import ml_dtypes
    f16 = np.float16
    f8 = ml_dtypes.float8_e4m3
    batch = np.asarray(batch, np.float32)
    W_qkv = np.asarray(W_qkv, np.float32)
    u_qkv = np.asarray(u_qkv, np.float32)
    sigma_qkv = np.asarray(sigma_qkv, np.float32)
    W_proj = np.asarray(W_proj, np.float32)
    u_proj = np.asarray(u_proj, np.float32)
    sigma_proj = np.asarray(sigma_proj, np.float32)
    c_qkv, c_proj = _host_scales(W_qkv, u_qkv, sigma_qkv,
                                 W_proj, u_proj, sigma_proj)
    xT = np.ascontiguousarray(batch.reshape(NTOK, D).T)
    xb_h = xT.astype(f16)
    x8_h = xT.astype(f8)
    # S needs scale c^2/sqrt(hd); gamma is the per-side share applied at the
    # q^T/k^T PSUM->SBUF copies (fp8 weights carry a BETA pre-scale to stay
    # in e4m3's normal range)
    # (gamma*BETA)^2 == c^2 / sqrt(hd)  =>  gamma = c / (BETA * hd^(1/4))
    gamma = np.float32(c_qkv / (BETA * HD ** 0.25))
    a = np.arange(128)
    triM = np.where(a[:, None] < a[None, :], -50.0, 0.0).astype(np.float32)
    ident = np.eye(128, dtype=np.float32)
    cst = np.concatenate(
        [np.full((128, 1), gamma, np.float32), triM, ident],
        axis=1).astype(f16)
    cst = np.ascontiguousarray(cst)
    in_maps = []
    for c in range(N_CORES):
        cs = slice(128 * c, 128 * (c + 1))
        w8q_h = np.ascontiguousarray((W_qkv[:, cs] * BETA).astype(f8))
        w8k_h = np.ascontiguousarray(
            (W_qkv[:, 1024 + 128 * c:1024 + 128 * (c + 1)] * BETA).astype(f8))
        wvh_h = np.ascontiguousarray(
            (W_qkv[:, 2048 + 128 * c:2048 + 128 * (c + 1)] * c_qkv)
            .astype(f16))
        wph_h = np.ascontiguousarray((W_proj[cs, :] * c_proj).astype(f16))
        in_maps.append({
            "xb": xb_h,
            "x8": x8_h,
            "w8qk": np.ascontiguousarray(
                np.concatenate([w8q_h, w8k_h], axis=1)),
            "wvh": wvh_h,
            "wph": wph_h,
            "cst": cst,
        })
    return in_maps


_NC_CACHE = None


def build_nc():
    global _NC_CACHE
    if _NC_CACHE is None:
        nc = bass.Bass("TRN2", target_bir_lowering=False, debug=False,
                       num_devices=N_CORES)
        with _TileContextSplit(nc) as tc:
            _build_body(nc, tc)
        _NC_CACHE = nc
    return _NC_CACHE


def kernel(batch, W_qkv, u_qkv, sigma_qkv, W_proj, u_proj, sigma_proj):
    in_maps = make_in_maps(batch, W_qkv, u_qkv, sigma_qkv,
                           W_proj, u_proj, sigma_proj)
    nc = build_nc()
    res = run_bass_kernel_spmd(nc, in_maps, list(range(N_CORES)))
    y = np.zeros((NTOK, D), np.float32)
    for c in range(N_CORES):
        y += res.results[c]["y"].astype(np.float32)
    return y.reshape(BATCH, NSEQ, D)
